# revision 3
# baseline (speedup 1.0000x reference)
"""Trainium2 Bass kernel for GroundTruthBasedPriorNetwork.

Per-node tiny MLP over a banded DAG, batched over 131072 samples:
    x[b, n, p]  = gt_labels[b, parent_idx[n, p]]          (N=64 nodes, P=8)
    h[b, n, :]  = tanh(W1[n] @ x[b, n, :] + b1[n])        (HID=16)
    mus[b, n]   = W2[n] . h[b, n, :] + b2[n]
    logvars     = zeros

Strategy: pure data parallel over 8 NeuronCores (batch split 8x16384).
The parent gather is folded on the host into a dense (64 x 1024) matrix
W1_full with W1_full[j, 16n+h] = sum_p [parent_idx[n,p]==j] W1[n,h,p],
augmented with a bias row (row 64) that multiplies a ones-row appended to
the host-transposed input XT (65 x 16384), all in bf16.

On-device per 256-column group g: 8 bf16 matmuls -> PSUM l1 (128x2048)
= h_pre^T, one Tanh activation (ScalarE, the bottleneck at ~2.0us/group)
-> SBUF bf16 h, 8 accumulating bf16 matmuls against block-structured W2T
into a (64x256) corner of the same l1 tile (its data is dead once the
Tanh has read it -- reusing it lets l1 double-buffer across the full 8
PSUM banks), then a DVE tensor_scalar add of b2 evacuates PSUM->SBUF in
bf16.  Emission is software-pipelined (L2 of group g-1 is emitted after
L1 of group g) so the PE never sits in program order behind a Tanh it
doesn't depend on.  Output is bf16 node-major (64 x 16384) per core,
un-transposed and cast to fp32 on the host.
"""

import os

import numpy as np

NUM_NODES = 64
MAX_P = 8
HID = 16
HFULL = NUM_NODES * HID  # 1024
BATCH = 131072
NCORES = 8
BC = BATCH // NCORES  # 16384 per core
GROUP = 256  # batch columns per group
NG = BC // GROUP  # 64 groups
OUT_CHUNK = 16  # groups per output DMA chunk

_COMPILED = {}


def _bf16(a):
    import ml_dtypes

    return np.asarray(a, np.float32).astype(ml_dtypes.bfloat16)


def _build_weights(W1, b1, W2, b2, parent_idx):
    """Host-side preprocessing of the tiny per-node weights."""
    W1 = np.asarray(W1, np.float32)
    b1 = np.asarray(b1, np.float32)
    W2 = np.asarray(W2, np.float32)
    b2 = np.asarray(b2, np.float32)
    parent_idx = np.asarray(parent_idx)

    # W1_full[j, 16n+h] = sum_p [parent_idx[n,p]==j] * W1[n,h,p]
    w1_full = np.zeros((NUM_NODES, HFULL), np.float32)
    for n in range(NUM_NODES):
        for p in range(MAX_P):
            j = int(parent_idx[n, p])
            w1_full[j, 16 * n : 16 * n + 16] += W1[n, :, p]
    w1_aug = np.concatenate([w1_full, b1.reshape(1, HFULL)], axis=0)  # (65, 1024)

    # W2T[p, 64t+n] = W2[n, hf%16] where hf = 128t+p and n == hf//16, else 0
    w2t = np.zeros((128, 8 * NUM_NODES), np.float32)
    for t in range(8):
        for p in range(128):
            hf = 128 * t + p
            n = hf // HID
            w2t[p, NUM_NODES * t + n] = W2[n, hf % HID]

    wpack = np.zeros((128, HFULL + 8 * NUM_NODES), np.float32)
    wpack[: NUM_NODES + 1, :HFULL] = w1_aug
    wpack[:, HFULL : HFULL + 8 * NUM_NODES] = w2t
    return _bf16(wpack), np.ascontiguousarray(b2.reshape(NUM_NODES, 1))


def _build_nc():
    import concourse.bacc as bacc
    import concourse.mybir as mybir
    import concourse.tile as tile
    from contextlib import ExitStack

    f32 = mybir.dt.float32
    bf16 = mybir.dt.bfloat16

    nc = bacc.Bacc("TRN2", target_bir_lowering=False, debug=False,
                   num_devices=NCORES)

    CW = HFULL + 8 * NUM_NODES  # 1536
    xt_d = nc.dram_tensor("xt", [NUM_NODES + 1, BC], bf16, kind="ExternalInput")
    wpack_d = nc.dram_tensor("wpack", [128, CW], bf16, kind="ExternalInput")
    b2_d = nc.dram_tensor("b2", [NUM_NODES, 1], f32, kind="ExternalInput")
    out_d = nc.dram_tensor("out", [NUM_NODES, BC], bf16, kind="ExternalOutput")

    NXT = 4  # number of xt sbuf tiles / input DMA chunks
    XTW = BC // NXT  # 4096 columns each
    NOC = NG // OUT_CHUNK  # 4 output chunks
    OCW = OUT_CHUNK * GROUP  # 4096

    with tile.TileContext(nc) as tc, ExitStack() as ctx:
        consts = ctx.enter_context(tc.tile_pool(name="consts", bufs=1))
        xt_pool = ctx.enter_context(tc.tile_pool(name="xt", bufs=1))
        out_pool = ctx.enter_context(tc.tile_pool(name="out", bufs=1))
        h_pool = ctx.enter_context(tc.tile_pool(name="h", bufs=2))
        l1_pool = ctx.enter_context(tc.tile_pool(name="l1", bufs=2, space="PSUM"))

        wpack_sb = consts.tile([128, CW], bf16, tag="wpack")
        b2_sb = consts.tile([NUM_NODES, 1], f32, tag="b2")
        nc.sync.dma_start(wpack_sb[:], wpack_d.ap())
        nc.sync.dma_start(b2_sb[:], b2_d.ap())
        w1_sb = wpack_sb[: NUM_NODES + 1, :HFULL]
        w2t_sb = wpack_sb[:, HFULL : HFULL + 8 * NUM_NODES]

        xt_tiles = []
        for k in range(NXT):
            xt_sb = xt_pool.tile(
                [NUM_NODES + 1, XTW], bf16, tag=f"xt{k}", name=f"xt_sb{k}"
            )
            xt_tiles.append(xt_sb)

        # Load the first batch chunk, then fence: the barrier absorbs the
        # const + first-chunk DMA waits so the matmuls (whose weight-load
        # micro-op has a tight sync-wait budget) carry at most a couple of
        # semaphore waits each.
        nc.sync.dma_start(xt_tiles[0][:], xt_d.ap()[:, 0:XTW])
        tc.strict_bb_all_engine_barrier()
        for k in range(1, NXT):
            nc.sync.dma_start(xt_tiles[k][:], xt_d.ap()[:, k * XTW : (k + 1) * XTW])

        out_tiles = [
            out_pool.tile([NUM_NODES, OCW], bf16, tag=f"out{k}", name=f"out_sb{k}")
            for k in range(NOC)
        ]

        l1_tiles = [None] * NG
        h_tiles = [None] * NG

        def emit_l1(g):
            xk, xoff = divmod(g * GROUP, XTW)
            rhs = xt_tiles[xk][:, xoff : xoff + GROUP]
            l1 = l1_pool.tile([128, 8 * GROUP], f32, tag="l1")
            l1_tiles[g] = l1
            for t in range(8):
                nc.tensor.matmul(
                    l1[:, t * GROUP : (t + 1) * GROUP],
                    w1_sb[:, t * 128 : (t + 1) * 128],
                    rhs,
                    start=True,
                    stop=True,
                )

        def emit_act(g):
            h = h_pool.tile([128, 8 * GROUP], bf16, tag="h")
            h_tiles[g] = h
            nc.scalar.activation(
                h[:], l1_tiles[g][:], mybir.ActivationFunctionType.Tanh
            )

        def emit_l2(g):
            # mus reuses the first PSUM bank of this group's (now consumed)
            # l1 tile, so l1 can double-buffer across all 8 PSUM banks.
            h = h_tiles[g]
            mus = l1_tiles[g][:NUM_NODES, :GROUP]
            for t in range(8):
                nc.tensor.matmul(
                    mus,
                    w2t_sb[:, t * NUM_NODES : (t + 1) * NUM_NODES],
                    h[:, t * GROUP : (t + 1) * GROUP],
                    start=(t == 0),
                    stop=(t == 7),
                )
            ok, ooff = divmod(g * GROUP, OCW)
            nc.vector.tensor_scalar_add(
                out_tiles[ok][:, ooff : ooff + GROUP], mus, b2_sb[:]
            )
            h_tiles[g] = None
            l1_tiles[g] = None
            if (g + 1) % OUT_CHUNK == 0:
                k = g // OUT_CHUNK
                nc.sync.dma_start(
                    out_d.ap()[:, k * OCW : (k + 1) * OCW], out_tiles[k][:]
                )

        # PE program order per group: L2(g-1) then L1(g).  The DVE evac of
        # mus(g-1) (which frees that l1 buffer) then overlaps L1(g) instead
        # of sitting between PE bursts on the critical path, and the PE
        # stays continuously busy so the HAM clock gate warms to 2.4 GHz.
        for g in range(NG):
            if g > 0:
                emit_l2(g - 1)
            emit_l1(g)
            emit_act(g)
        emit_l2(NG - 1)

    nc.finalize()
    return nc


def _get_nc():
    if "nc" not in _COMPILED:
        _COMPILED["nc"] = _build_nc()
    return _COMPILED["nc"]


def kernel(gt_labels, W1, b1, W2, b2, parent_idx):
    import ml_dtypes
    from concourse.bass_utils import run_bass_kernel_spmd

    gt_labels = np.asarray(gt_labels, np.float32)
    wpack, b2c = _build_weights(W1, b1, W2, b2, parent_idx)

    in_maps = []
    for c in range(NCORES):
        xc = gt_labels[c * BC : (c + 1) * BC]  # (16384, 64)
        xt = np.empty((NUM_NODES + 1, BC), ml_dtypes.bfloat16)
        xt[:NUM_NODES] = xc.T.astype(ml_dtypes.bfloat16)
        xt[NUM_NODES] = 1.0
        in_maps.append({"xt": xt, "wpack": wpack, "b2": b2c})

    nc = _get_nc()
    trace = bool(int(os.environ.get("KERNEL_TRACE", "0")))
    res = run_bass_kernel_spmd(nc, in_maps, list(range(NCORES)), trace=trace)
    if trace and res.exec_time_ns is not None:
        print(f"HW exec time: {res.exec_time_ns} ns")
        _COMPILED["exec_time_ns"] = res.exec_time_ns

    mus = np.empty((BATCH, NUM_NODES), np.float32)
    for c in range(NCORES):
        mus[c * BC : (c + 1) * BC] = res.results[c]["out"].astype(np.float32).T
    mus = mus.reshape(BATCH, NUM_NODES, 1)
    logvars = np.zeros_like(mus)
    return mus, logvars
